# revision 15
# baseline (speedup 1.0000x reference)
"""Trainium2 Bass kernel for a GPT-style transformer block (B=4, T=1024, C=1024, H=16).

Sharding: 8 cores = (batch b in 0..3) x (sequence half h in 0..1). Each core
computes the full block for its 512 "own" tokens; K/V are computed redundantly
over all 1024 tokens of its batch, so there is no cross-core communication.
Per-core token order is rolled so own tokens are always columns 0:512 — the
SPMD program is identical on every core, only the input data differs.

v2: attention is a per-head-pair software pipeline
(S(h) | AV(h-1) | Q(h+1) | K(h+1) interleaved per key-chunk-pair) so the
scalar engine's exp stream overlaps tensor-engine matmuls instead of
serializing after QKV. Exp runs on [128,1024] chunks (the ~450ns per-activate
fixed cost made [128,512] chunks 48% more expensive in v1). Score matmuls for
the two heads of a pair sit at partitions 0:64 / 64:128 and are emitted
adjacently -> the PE runs them concurrently in distinct row groups. LayerNorm
normalize ops are interleaved with the first consumer's matmuls (Q0/K0, fc
group 0) to keep PE idle gaps under the ~3.4us HAM re-throttle window.
PSUM: ps_s = 2x[128,1024] (4 banks), ps_w = 4x[128,512] (4 banks).
"""

import numpy as np
import ml_dtypes

import concourse.bass as bass
import concourse.bacc as bacc
import concourse.tile as tile
import concourse.mybir as mybir
from concourse.bass_utils import run_bass_kernel_spmd

P = 128
B, T, C, H, D = 4, 1024, 1024, 16, 64
KO = C // P          # 8 contraction chunks of 128 channels
TOWN = T // 2        # 512 own tokens per core
FF = 4 * C

F32 = mybir.dt.float32
BF16 = mybir.dt.bfloat16
np_bf16 = ml_dtypes.bfloat16

Alu = mybir.AluOpType
Act = mybir.ActivationFunctionType

# set by kernel() so an external harness (test.py) can read trace results
TRACE = False
TRACE_KW = {}
LAST_RESULTS = None
_NC_CACHE = None


def _emit(nc, tc, io):
    from contextlib import ExitStack

    T2 = 2 * TOWN
    with ExitStack() as ctx:
        ep = ctx.enter_context
        consts = ep(tc.tile_pool(name="consts", bufs=1))
        p_wqk = ep(tc.tile_pool(name="p_wqk", bufs=4))
        p_wv = ep(tc.tile_pool(name="p_wv", bufs=9))
        p_wcp = ep(tc.tile_pool(name="p_wcp", bufs=3))
        p_wfc = ep(tc.tile_pool(name="p_wfc", bufs=5))
        p_wpj = ep(tc.tile_pool(name="p_wpj", bufs=4))
        p_big = ep(tc.tile_pool(name="p_big", bufs=3))    # x_bf / xln / x2 / h0 / h1
        p_act = ep(tc.tile_pool(name="p_act", bufs=1))    # persistent activations
        p_scr = ep(tc.tile_pool(name="p_scr", bufs=2))    # [P, TOWN] bf16 scratch
        p_pt = ep(tc.tile_pool(name="p_pt", bufs=12))     # exp(S^T) chunks [P,T2]
        p_row = ep(tc.tile_pool(name="p_row", bufs=2))    # [1, T] stat rows
        p_zrow = ep(tc.tile_pool(name="p_zrow", bufs=3))  # [1, TOWN] rows
        p_bc = ep(tc.tile_pool(name="p_bc", bufs=2))      # [P, TOWN] f32 bcast
        p_out = ep(tc.tile_pool(name="p_out", bufs=2))    # output staging
        # PSUM: ps_s = 2 x [P, T2] (4 banks), ps_y / ps_w = 2 x [P, TOWN] each.
        # ps_y holds the long-lived AV accumulators (and doubles the fc/c_proj
        # chains); ps_w cycles the short Q/K/V/fc/proj chains. Keeping them in
        # separate rings avoids a round-robin wrap onto a live AV tile.
        ps_s = ep(tc.tile_pool(name="ps_s", bufs=2, space="PSUM"))
        ps_y = ep(tc.tile_pool(name="ps_y", bufs=2, space="PSUM"))
        ps_w = ep(tc.tile_pool(name="ps_w", bufs=2, space="PSUM"))

        def psS():
            return ps_s.tile([P, T2], F32, tag="s", name="pss")

        def psY():
            return ps_y.tile([P, TOWN], F32, tag="y", name="psy")

        def psW():
            return ps_w.tile([P, TOWN], F32, tag="w", name="psw")

        # ---- constants ----
        ones_mean_bf = consts.tile([P, 1], BF16)    # 1/C  -> ones-matmul = mean
        nc.vector.memset(ones_mean_bf, 1.0 / C)
        ones_row = consts.tile([1, P], F32)         # 1.0  -> partition broadcast matmul
        nc.vector.memset(ones_row, 1.0)

        # ---- x^T bf16 first on sync+gpsimd queues (residual = own half) ----
        x_bf = p_big.tile([P, KO, T], BF16, tag="big")
        for ko in range(KO):
            (nc.sync if ko % 2 == 0 else nc.gpsimd).dma_start(
                out=x_bf[:, ko, :], in_=io["x_bf"][:, ko, :])

        # Q/K weight prefetch: head pair hp uses wqk[hp] (Q) and wqk[8+hp] (K)
        def fetch_wqk(hp):
            tiles = []
            for j, mo in enumerate((hp, 8 + hp)):
                wt = p_wqk.tile([P, KO, P], BF16, tag="wqk")
                (nc.sync if j == 0 else nc.gpsimd).dma_start(out=wt, in_=io["wqk"][mo])
                tiles.append(wt)
            return tiles

        wqk_f = {0: fetch_wqk(0), 1: fetch_wqk(1)}

        # biases / mask on the scalar engine's DMA queue (idle at startup)
        bqk_sb = consts.tile([P, 16], F32)
        nc.scalar.dma_start(out=bqk_sb, in_=io["bqk"][:])
        ncs_sb = consts.tile([P, 16], F32)
        nc.scalar.dma_start(out=ncs_sb, in_=io["ncs"][:])
        bcp_sb = consts.tile([P, KO], F32)
        nc.scalar.dma_start(out=bcp_sb, in_=io["bcp"][:])
        bfc_sb = consts.tile([P, 32], F32)
        nc.scalar.dma_start(out=bfc_sb, in_=io["bfc"][:])
        bpj_sb = consts.tile([P, KO], F32)
        nc.scalar.dma_start(out=bpj_sb, in_=io["bpj"][:])
        ebias_sb = consts.tile([P, 1], F32)
        nc.scalar.dma_start(out=ebias_sb, in_=io["ebias"][:])
        mask_sb = p_act.tile([P, 4, TOWN], BF16, tag="mask")   # per-kc tril chunks
        nc.scalar.dma_start(out=mask_sb, in_=io["mask"][:])
        bv_sb = consts.tile([P, C], BF16)
        nc.scalar.dma_start(out=bv_sb, in_=io["bv"][:])

        # ---- LayerNorm 1 stats (bf16 ones-matmuls; mu|sq chains share a tile) ----
        st = [psS(), psS()]          # [0:1, 0:TOWN]=mu, [0:1, TOWN:T2]=sq per half
        for ko in range(KO):
            sq_t = []
            for half in range(2):
                sq = p_scr.tile([P, TOWN], BF16, tag="scr")
                nc.vector.tensor_mul(sq, x_bf[:, ko, half * TOWN:(half + 1) * TOWN],
                                     x_bf[:, ko, half * TOWN:(half + 1) * TOWN])
                sq_t.append(sq)
            for half in range(2):
                xb = x_bf[:, ko, half * TOWN:(half + 1) * TOWN]
                nc.tensor.matmul(st[half][0:1, 0:TOWN], ones_mean_bf, xb,
                                 start=(ko == 0), stop=(ko == KO - 1))
                nc.tensor.matmul(st[half][0:1, TOWN:T2], ones_mean_bf, sq_t[half],
                                 start=(ko == 0), stop=(ko == KO - 1))

        # rows: mu | rstd over the full T tokens (all at partition 0)
        mu_row = p_row.tile([1, T], F32, tag="row")
        rs_row = p_row.tile([1, T], F32, tag="row")
        for half in range(2):
            sl = slice(half * TOWN, (half + 1) * TOWN)
            nc.vector.tensor_copy(mu_row[0:1, sl], st[half][0:1, 0:TOWN])
            tv = p_zrow.tile([1, TOWN], F32, tag="zrow")
            nc.vector.tensor_mul(tv, mu_row[0:1, sl], mu_row[0:1, sl])
            nc.vector.tensor_sub(rs_row[0:1, sl], st[half][0:1, TOWN:T2], tv)
        nc.scalar.activation(rs_row, rs_row, Act.Sqrt)
        nc.vector.tensor_scalar_add(rs_row, rs_row, 1e-5)
        nc.vector.reciprocal_approx_fast(rs_row, rs_row)

        # ---- persistent attention tensors ----
        xln = p_big.tile([P, KO, T], BF16, tag="big")
        qT = p_act.tile([P, KO, TOWN], BF16, tag="qT")
        kT = p_act.tile([P, KO, T], BF16, tag="kT")
        v_ext = p_act.tile([P, KO, 16 * 65], BF16, tag="v")
        v_r = v_ext.rearrange("p k (h d) -> p k h d", d=65)
        nc.vector.memset(v_r[:, :, :, 64:65], 1.0)   # ones column -> softmax denom
        yT = p_act.tile([P, KO, TOWN], BF16, tag="yT")

        def norm_chunk(ko, half):
            sl = slice(half * TOWN, (half + 1) * TOWN)
            tmp = p_scr.tile([P, TOWN], BF16, tag="scr")
            nc.vector.tensor_sub(tmp, x_bf[:, ko, sl], mu_bf[:, sl])
            nc.vector.tensor_mul(xln[:, ko, sl], tmp, rs_bf[:, sl])

        def evict_q(hp, ps):
            tmp = p_scr.tile([P, TOWN], BF16, tag="scr", name="evtmp")
            nc.vector.scalar_tensor_tensor(
                tmp, mu_bf[:, 0:TOWN], ncs_sb[:, hp:hp + 1], ps,
                op0=Alu.mult, op1=Alu.add)
            nc.vector.scalar_tensor_tensor(
                qT[:, hp, :], tmp, bqk_sb[:, hp:hp + 1], rs_bf[:, 0:TOWN],
                op0=Alu.add, op1=Alu.mult)

        def evict_k(hp, pss):
            for half in range(2):
                sl = slice(half * TOWN, (half + 1) * TOWN)
                tmp = p_scr.tile([P, TOWN], BF16, tag="scr", name="evtmp")
                nc.vector.scalar_tensor_tensor(
                    tmp, mu_bf[:, sl], ncs_sb[:, 8 + hp:9 + hp], pss[half],
                    op0=Alu.mult, op1=Alu.add)
                nc.vector.scalar_tensor_tensor(
                    kT[:, hp, sl], tmp, bqk_sb[:, 8 + hp:9 + hp], rs_bf[:, sl],
                    op0=Alu.add, op1=Alu.mult)

        def emit_q(hp, wt):
            ps = psW()
            for ko in range(KO):
                nc.tensor.matmul(ps, wt[:, ko, :], x_bf[:, ko, 0:TOWN],
                                 start=(ko == 0), stop=(ko == KO - 1))
            evict_q(hp, ps)

        def emit_k(hp, wt):
            pss = [psW(), psW()]
            for ko in range(KO):
                for half in range(2):
                    nc.tensor.matmul(pss[half], wt[:, ko, :],
                                     x_bf[:, ko, half * TOWN:(half + 1) * TOWN],
                                     start=(ko == 0), stop=(ko == KO - 1))
            evict_k(hp, pss)

        # raw Q0/K0 matmuls right after the stats chains (no LN dependency);
        # the mu/rstd broadcast runs on the PE behind them, then the folded
        # evictions fire. q0 lives in ps_y so k0's two ps_w chains don't wrap
        # onto an un-evicted tile.
        q0_ps = psY()
        for ko in range(KO):
            nc.tensor.matmul(q0_ps, wqk_f[0][0][:, ko, :], x_bf[:, ko, 0:TOWN],
                             start=(ko == 0), stop=(ko == KO - 1))
        k0_ps = [psW(), psW()]
        for ko in range(KO):
            for half in range(2):
                nc.tensor.matmul(k0_ps[half], wqk_f[0][1][:, ko, :],
                                 x_bf[:, ko, half * TOWN:(half + 1) * TOWN],
                                 start=(ko == 0), stop=(ko == KO - 1))

        # broadcast mu/rstd to all partitions, evict to bf16 for 2x DVE rate
        mu_bf = p_act.tile([P, T], BF16, tag="mubf")
        rs_bf = p_act.tile([P, T], BF16, tag="rsbf")
        for half in range(2):
            sl = slice(half * TOWN, (half + 1) * TOWN)
            bc = psS()
            nc.tensor.matmul(bc[:, 0:TOWN], ones_row, mu_row[0:1, sl],
                             start=True, stop=True)
            nc.tensor.matmul(bc[:, TOWN:T2], ones_row, rs_row[0:1, sl],
                             start=True, stop=True)
            nc.scalar.copy(mu_bf[:, sl], bc[:, 0:TOWN])
            nc.vector.tensor_copy(rs_bf[:, sl], bc[:, TOWN:T2])

        evict_q(0, q0_ps)
        evict_k(0, k0_ps)
        # xln (only consumed by the V projection) normalizes in the background
        for ko in range(KO):
            norm_chunk(ko, 0)

        all_pts = {}

        def emit_s_block(hp, kc):
            """Both heads' scores for one key chunk share a psum tile, so the
            row-group-0/64 matmuls release together and truly pair."""
            ps = psS()
            for i in range(2):
                pb = 64 * i
                nc.tensor.matmul(ps[:, i * TOWN:(i + 1) * TOWN],
                                 kT[pb:pb + 64, hp, kc * P:(kc + 1) * P],
                                 qT[pb:pb + 64, hp, :], start=True, stop=True)
            pt = p_pt.tile([P, T2], BF16, tag="pt")
            if kc < 4:
                nc.scalar.activation(pt, ps, Act.Exp)
                for i in range(2):
                    nc.vector.tensor_mul(pt[:, i * TOWN:(i + 1) * TOWN],
                                         pt[:, i * TOWN:(i + 1) * TOWN],
                                         mask_sb[:, kc, :])
            else:
                nc.scalar.activation(pt, ps, Act.Exp, bias=ebias_sb[:, 0:1])
            all_pts[(hp, kc)] = pt

        def emit_av_block(hp, psy, kc):
            pt = all_pts.pop((hp, kc))
            for i in range(2):
                hd = 2 * hp + i
                nc.tensor.matmul(psy[i][0:65, :],
                                 v_ext[:, kc, hd * 65:(hd + 1) * 65],
                                 pt[:, i * TOWN:(i + 1) * TOWN],
                                 start=(kc == 0), stop=(kc == KO - 1))

        def finish_av(hp, psy):
            for i in range(2):
                pb = 64 * i
                z = p_zrow.tile([1, TOWN], F32, tag="zrow")
                nc.vector.tensor_copy(z, psy[i][64:65, :])
                rz = p_zrow.tile([1, TOWN], F32, tag="zrow")
                nc.vector.reciprocal_approx_fast(rz, z)
                rzbc = p_bc.tile([P, TOWN], F32, tag="bc")
                nc.gpsimd.partition_broadcast(rzbc, rz, channels=P)
                nc.vector.tensor_mul(yT[pb:pb + 64, hp, :], psy[i][0:64, :],
                                     rzbc[0:64, :])

        def fetch_wcp(mop):
            tiles = []
            for half in range(2):
                mo = 2 * mop + half
                wt = p_wcp.tile([P, KO, P], BF16, tag="wcp", name="wcpt")
                (nc.sync if half == 0 else nc.gpsimd).dma_start(
                    out=wt, in_=io["wcp"][mo])
                tiles.append(wt)
            return tiles

        # V weight prefetch for nh=0 (used right after the h=0 score iteration)
        def fetch_wv(nh):
            tiles = []
            for ko in range(KO):
                w = p_wv.tile([P, TOWN], BF16, tag="wv")
                (nc.sync if ko % 2 == 0 else nc.gpsimd).dma_start(
                    out=w, in_=io["wv"][ko, nh])
                tiles.append(w)
            return tiles

        wv_tiles = fetch_wv(0)

        # h=0 iteration: S(0) with Q(1)/K(1) as PE filler (no AV yet)
        for kc in range(KO):
            emit_s_block(0, kc)
            if kc == 0:
                emit_q(1, wqk_f[1][0])
                for ko in range(KO):
                    norm_chunk(ko, 1)
            elif kc == 2:
                emit_k(1, wqk_f[1][1])
            elif kc == 4:
                wqk_f[2] = fetch_wqk(2)

        # ---- V projection (natural layout, token chunks on partitions) ----
        for nh in range(2):
            wvt = wv_tiles
            if nh == 0:
                wv_tiles = fetch_wv(1)
            for tkb in range(KO):
                ps = psW()
                for ko in range(KO):
                    nc.tensor.matmul(ps, xln[:, ko, tkb * P:(tkb + 1) * P],
                                     wvt[ko], start=(ko == 0), stop=(ko == KO - 1))
                vout = v_r[:, tkb]
                nc.vector.tensor_add(
                    vout[:, nh * 8:(nh + 1) * 8, 0:64],
                    ps.rearrange("p (h d) -> p h d", d=64),
                    bv_sb[:, nh * TOWN:(nh + 1) * TOWN].rearrange(
                        "p (h d) -> p h d", d=64))

        # ---- per-head pipeline: S(h) | AV(h-1) | Q(h+1)/K(h+1) ----
        for h in range(1, 9):
            if h < 8:
                psy = [psY(), psY()]
                for kc in range(KO):
                    emit_s_block(h, kc)
                    emit_av_block(h - 1, psy, kc)
                    if h < 7:
                        if kc == 0:
                            emit_q(h + 1, wqk_f[h + 1][0])
                        elif kc == 2:
                            emit_k(h + 1, wqk_f[h + 1][1])
                        elif kc == 4 and h + 2 <= 7:
                            wqk_f[h + 2] = fetch_wqk(h + 2)
                finish_av(h - 1, psy)
            else:
                wcp_next = fetch_wcp(0)
                psy = [psY(), psY()]
                for kc in range(KO):
                    emit_av_block(7, psy, kc)
                finish_av(7, psy)

        # ---- c_proj + residual -> x2 (bf16) with LN2 stats interleaved ----
        x2c = p_big.tile([P, KO, T], BF16, tag="big")   # [.. 0:TOWN]=x2, [TOWN:]=x2ln
        x2_bf = x2c[:, :, 0:TOWN]
        x2ln = x2c[:, :, TOWN:T2]
        st2 = psS()          # [0:1, 0:TOWN]=mu, [0:1, TOWN:T2]=sq
        for mop in range(4):
            wts = wcp_next
            if mop < 3:
                wcp_next = fetch_wcp(mop + 1)
            pss = [psY(), psW()]
            for ko in range(KO):
                for half in range(2):
                    nc.tensor.matmul(pss[half], wts[half][:, ko, :], yT[:, ko, :],
                                     start=(ko == 0), stop=(ko == KO - 1))
            for half in range(2):
                mo = 2 * mop + half
                nc.vector.scalar_tensor_tensor(
                    x2_bf[:, mo, :], pss[half], bcp_sb[:, mo:mo + 1],
                    x_bf[:, mo, 0:TOWN], op0=Alu.add, op1=Alu.add)
                # LN2 stats on the fresh chunk
                sq = p_scr.tile([P, TOWN], BF16, tag="scr")
                nc.vector.tensor_mul(sq, x2_bf[:, mo, :], x2_bf[:, mo, :])
                nc.tensor.matmul(st2[0:1, 0:TOWN], ones_mean_bf, x2_bf[:, mo, :],
                                 start=(mo == 0), stop=(mo == KO - 1))
                nc.tensor.matmul(st2[0:1, TOWN:T2], ones_mean_bf, sq,
                                 start=(mo == 0), stop=(mo == KO - 1))

        # LN2 tail
        mu2_row = p_zrow.tile([1, TOWN], F32, tag="zrow")
        nc.vector.tensor_copy(mu2_row, st2[0:1, 0:TOWN])
        t2v = p_zrow.tile([1, TOWN], F32, tag="zrow")
        nc.vector.tensor_mul(t2v, mu2_row, mu2_row)
        rs2_row = p_zrow.tile([1, TOWN], F32, tag="zrow")
        nc.vector.tensor_sub(rs2_row, st2[0:1, TOWN:T2], t2v)
        nc.scalar.activation(rs2_row, rs2_row, Act.Sqrt)
        nc.vector.tensor_scalar_add(rs2_row, rs2_row, 1e-5)
        nc.vector.reciprocal_approx_fast(rs2_row, rs2_row)

        mu2_bf = p_scr.tile([P, TOWN], BF16, tag="scr")
        rs2_bf = p_scr.tile([P, TOWN], BF16, tag="scr")
        bc2 = psS()
        nc.tensor.matmul(bc2[:, 0:TOWN], ones_row, mu2_row, start=True, stop=True)
        nc.tensor.matmul(bc2[:, TOWN:T2], ones_row, rs2_row, start=True, stop=True)
        nc.scalar.copy(mu2_bf, bc2[:, 0:TOWN])
        nc.vector.tensor_copy(rs2_bf, bc2[:, TOWN:T2])

        def norm2_chunk(ko):
            tmp = p_scr.tile([P, TOWN], BF16, tag="scr2")
            nc.vector.tensor_sub(tmp, x2_bf[:, ko, :], mu2_bf)
            nc.vector.tensor_mul(x2ln[:, ko, :], tmp, rs2_bf)

        # ---- MLP fc (+gelu): 16 groups of 2 output chunks, ko-outer.
        # Group 0 interleaves the LN2 normalize so the PE doesn't idle.
        h0 = p_big.tile([P, KO, T], BF16, tag="big", name="h0").rearrange(
            "p a (b c) -> p (a b) c", c=TOWN)      # [P, 16, TOWN]
        h1 = p_big.tile([P, KO, T], BF16, tag="big", name="h1").rearrange(
            "p a (b c) -> p (a b) c", c=TOWN)
        hh = [h0, h1]
        for g in range(16):
            wts = []
            for m in range(2):
                mo = 2 * g + m
                wt = p_wfc.tile([P, KO, P], BF16, tag="wfc")
                (nc.sync if mo % 2 == 0 else nc.gpsimd).dma_start(
                    out=wt, in_=io["wfc"][mo])
                wts.append(wt)
            chains = [psY(), psW()]
            for ko in range(KO):
                if g == 0:
                    norm2_chunk(ko)
                for m in range(2):
                    nc.tensor.matmul(chains[m], wts[m][:, ko, :], x2ln[:, ko, :],
                                     start=(ko == 0), stop=(ko == KO - 1))
            for m in range(2):
                mo = 2 * g + m
                nc.scalar.activation(hh[mo // 16][:, mo % 16, :], chains[m],
                                     Act.Gelu, bias=bfc_sb[:, mo:mo + 1])

        # ---- MLP proj + residual -> out ----
        for mo in range(KO):
            wts = []
            for whalf in range(2):
                wt = p_wpj.tile([P, 16, P], BF16, tag="wpj")
                (nc.sync if whalf == 0 else nc.gpsimd).dma_start(
                    out=wt, in_=io["wpj"][mo][:, whalf * 16:(whalf + 1) * 16, :])
                wts.append(wt)
            chain = psW()
            for ko in range(32):
                nc.tensor.matmul(chain, wts[ko // 16][:, ko % 16, :],
                                 hh[ko // 16][:, ko % 16, :],
                                 start=(ko == 0), stop=(ko == 31))
            ot = p_out.tile([P, TOWN], F32, tag="outst")
            nc.vector.scalar_tensor_tensor(ot, chain, bpj_sb[:, mo:mo + 1],
                                           x2_bf[:, mo, :],
                                           op0=Alu.add, op1=Alu.add)
            nc.sync.dma_start(out=io["out"][:, mo, :], in_=ot)


def _build_nc():
    nc = bacc.Bacc("TRN2", target_bir_lowering=False, debug=False)
    io = {}
    dt = nc.dram_tensor
    io["x_bf"] = dt("x_bf", [P, KO, T], BF16, kind="ExternalInput")
    io["wqk"] = dt("wqk", [16, P, KO, P], BF16, kind="ExternalInput")
    io["wv"] = dt("wv", [KO, 2, P, TOWN], BF16, kind="ExternalInput")
    io["wcp"] = dt("wcp", [KO, P, KO, P], BF16, kind="ExternalInput")
    io["wfc"] = dt("wfc", [32, P, KO, P], BF16, kind="ExternalInput")
    io["wpj"] = dt("wpj", [KO, P, 32, P], BF16, kind="ExternalInput")
    io["bqk"] = dt("bqk", [P, 16], F32, kind="ExternalInput")
    io["ncs"] = dt("ncs", [P, 16], F32, kind="ExternalInput")
    io["bv"] = dt("bv", [P, C], BF16, kind="ExternalInput")
    io["bcp"] = dt("bcp", [P, KO], F32, kind="ExternalInput")
    io["bfc"] = dt("bfc", [P, 32], F32, kind="ExternalInput")
    io["bpj"] = dt("bpj", [P, KO], F32, kind="ExternalInput")
    io["mask"] = dt("mask", [P, 4, TOWN], BF16, kind="ExternalInput")
    io["ebias"] = dt("ebias", [P, 1], F32, kind="ExternalInput")
    io["out"] = dt("out", [P, KO, TOWN], F32, kind="ExternalOutput")
    with tile.TileContext(nc) as tc:
        _emit(nc, tc, io)
    nc.compile()
    return nc


def _prep_maps(inputs):
    f32 = np.float32
    g = {k: np.asarray(v, f32) for k, v in inputs.items()}

    # fold LN gains/biases into the following projections
    Wa = g["c_attn_w"] * g["ln1_w"][:, None]
    ba = g["c_attn_b"] + g["ln1_b"] @ g["c_attn_w"]
    Wq, Wk, Wv = Wa[:, :C] * 0.125, Wa[:, C:2 * C], Wa[:, 2 * C:]
    bq, bk, bv = ba[:C] * 0.125, ba[C:2 * C], ba[2 * C:]
    Wfc = g["fc_w"] * g["ln2_w"][:, None]
    bfc = g["fc_b"] + g["ln2_b"] @ g["fc_w"]

    def lhsT_arrange(w, n_mo):  # [C_in, N] -> [n_mo, P(ki), KO_in, P(mi)] bf16
        ko_in = w.shape[0] // P
        return np.ascontiguousarray(
            w.reshape(ko_in, P, n_mo, P).transpose(2, 1, 0, 3)).astype(np_bf16)

    shared = {
        "wqk": lhsT_arrange(np.concatenate([Wq, Wk], axis=1), 16),
        "wv": np.ascontiguousarray(
            Wv.reshape(KO, P, 2, TOWN).transpose(0, 2, 1, 3)).astype(np_bf16),
        "wcp": lhsT_arrange(g["c_proj_w"], KO),
        "wfc": lhsT_arrange(Wfc, 32),
        "wpj": lhsT_arrange(g["proj_w"], KO),
        "bqk": np.ascontiguousarray(
            np.concatenate([bq, bk]).reshape(16, P).T).astype(f32),
        "ncs": np.ascontiguousarray(
            -np.concatenate([Wq, Wk], axis=1).sum(axis=0).reshape(16, P).T
        ).astype(f32),
        "bv": np.ascontiguousarray(np.broadcast_to(bv, (P, C))).astype(np_bf16),
        "bcp": np.ascontiguousarray(g["c_proj_b"].reshape(KO, P).T).astype(f32),
        "bfc": np.ascontiguousarray(bfc.reshape(32, P).T).astype(f32),
        "bpj": np.ascontiguousarray(g["proj_b"].reshape(KO, P).T).astype(f32),
    }

    maps = []
    gq_base = np.arange(TOWN)
    gk_base = np.arange(T)
    for c in range(8):
        b, h = divmod(c, 2)
        xr = np.roll(g["x"][b], -h * TOWN, axis=0)          # own tokens first
        arr = np.ascontiguousarray(
            xr.T.reshape(KO, P, T).transpose(1, 0, 2)).astype(f32)  # [P, KO, T]
        m = (gk_base[:TOWN, None] <= gq_base[None, :]).astype(f32)  # tril [TOWN, TOWN]
        # [P(ki), kc, q] with key = kc*P + ki
        mask = np.ascontiguousarray(
            m.reshape(4, P, TOWN).transpose(1, 0, 2)).astype(np_bf16)
        ebias = np.full((P, 1), -50.0 if h == 0 else 0.0, f32)
        maps.append(dict(shared,
                         x_bf=arr.astype(np_bf16),
                         mask=mask, ebias=ebias))
    return maps


def kernel(**inputs):
    global LAST_RESULTS, _NC_CACHE
    if _NC_CACHE is None:
        _NC_CACHE = _build_nc()
    nc = _NC_CACHE
    maps = _prep_maps(inputs)
    res = run_bass_kernel_spmd(nc, maps, core_ids=list(range(8)),
                               trace=TRACE, **TRACE_KW)
    LAST_RESULTS = res
    out = np.zeros((B, T, C), np.float32)
    for c in range(8):
        b, h = divmod(c, 2)
        ot = res.results[c]["out"]                # [P, KO, TOWN]
        out[b, h * TOWN:(h + 1) * TOWN, :] = \
            ot.transpose(1, 0, 2).reshape(C, TOWN).T
    return out
